# revision 31
# baseline (speedup 1.0000x reference)
"""Trainium2 Bass kernel for the 4-directional Mamba (SS2D / VMamba-style)
block from the OSS reference.

Sharding: the 8 independent (direction x batch) sequences map one-per-core
(SPMD: one NEFF, 8 cores, per-core inputs). Backward directions are handled by
host-side flips of the input/output sequences; the final sum of the four
directional outputs plus the residual x2 happens at gather time on host.

Per-core kernel (C=96, L=4096, P=192, N=16, dtr=6):
  - causal depthwise conv folded into the input projection as 4 shifted
    tap-matmuls accumulating in PSUM (PE, fp16 operands)
  - x/z gates use the Silu ACT table straight out of PSUM; dt uses
    softplus(v) = -ln(sigmoid(-v)) via the Sigmoid + Ln tables, storing -dt
    (sign folded into the dA scale and the u multiply)
  - dA_n = Exp(acol[:, n] * (-dt)) via ACT per-partition scale columns
  - the selective scan runs on DVE's tensor_tensor_scan (DVE-only in HW);
    most hc multiplies run on Pool as plain tensor_tensor ops (the only
    two-tensor form GPSIMD supports) to keep DVE free for the scans; the
    P=192 state rows are packed as a full 128-row group plus a pair-packed
    64-row group
  - B/C rows round-trip through DRAM in an interleaved [n, sec, B|C, col]
    layout so each item's broadcast DMA is one 4KB-contiguous-run transfer
  - Dp is folded into a second output weight (woT2) consuming xz = x*silu(z),
    which removes the per-block yg pass
  - phase C of each section is deferred into the next section's item loop so
    it does not stall the dBx/scan pipeline; output copies run on ACT (Copy
    works under every table); GPSIMD never touches PSUM (illegal in HW)
  - phases are pipelined by L-halves: half 1's Silu/Sigmoid/Ln phase A
    overlaps half 0's scan work (3 ACT table loads per half)
"""

import numpy as np

C = 96
L = 4096
P = 192
PLO = 128
PHI = 64
N = 16
DTR = 6
DC = 4
HH = 64
WW = 64
MCH = 512
NSEC = 4
SEC = L // NSEC

_CACHED = {}


def _build_program(repeat=1, n_bufs=6, hc_pool=20, dbx_pool=0, halves=2,
                   bc_bufs=8, interleave_c=2, osb_act=True, proj_act=False,
                   u_pool=False, xz_pool=True, pool_stt=False):
    """The scan is DVE-only in hardware; hc_pool / dbx_pool give the number
    of items (of 24, taken from the end of the item order) whose hc / dBx
    multiply runs on Pool instead of DVE. pool_stt picks the Pool multiply
    form (scalar_tensor_tensor is NOT supported by GPSIMD in HW, so it must
    stay False). interleave_c: item index in the next section at which the
    previous section's phase C is emitted (None/-1 = emit immediately)."""
    from contextlib import ExitStack

    import concourse.bacc as bacc
    import concourse.bass as bass
    import concourse.tile as tile
    from concourse import mybir

    f32 = mybir.dt.float32
    f16 = mybir.dt.float16
    Alu = mybir.AluOpType
    Act = mybir.ActivationFunctionType

    nc = bacc.Bacc()

    seqT = nc.dram_tensor("seqT", [C, L], f16, kind="ExternalInput")
    wc = nc.dram_tensor("wc", [C, DC, P], f16, kind="ExternalInput")
    wz = nc.dram_tensor("wz", [C, P], f16, kind="ExternalInput")
    wdtT = nc.dram_tensor("wdtT", [DTR, P], f16, kind="ExternalInput")
    # packed per-partition params: [-b_dt | conv_b | -A] as [P, 2+N] f32
    wsc = nc.dram_tensor("wsc", [P, 2 + N], f32, kind="ExternalInput")
    # packed f16 weights: [wxT | woT | woT2 | idsel] on 128 partitions and
    # [wxT | woT | woT2] on the hi 64 partitions
    W0 = DTR + 2 * N
    wbig0 = nc.dram_tensor("wbig0", [PLO, W0 + 2 * C + PLO + PHI], f16,
                           kind="ExternalInput")
    wbig1 = nc.dram_tensor("wbig1", [PHI, W0 + 2 * C], f16,
                           kind="ExternalInput")
    out = nc.dram_tensor("out", [C, L], f32, kind="ExternalOutput")

    with tile.TileContext(nc) as tc, ExitStack() as ctx:
        wpool = ctx.enter_context(tc.tile_pool(name="weights", bufs=1))
        spool = ctx.enter_context(tc.tile_pool(name="seq", bufs=1))
        big_pool = ctx.enter_context(tc.tile_pool(name="big", bufs=1))
        tmp_pool = ctx.enter_context(tc.tile_pool(name="tmp", bufs=2))
        n_pool = ctx.enter_context(tc.tile_pool(name="nl", bufs=n_bufs))
        bc_pool = ctx.enter_context(tc.tile_pool(name="bc", bufs=bc_bufs))
        ps_pool = ctx.enter_context(tc.tile_pool(name="ps", bufs=2, space="PSUM"))
        ya0_pool = ctx.enter_context(
            tc.tile_pool(name="yps0", bufs=2, space="PSUM"))
        ya1_pool = ctx.enter_context(
            tc.tile_pool(name="yps1", bufs=1, space="PSUM"))
        dram_pool = ctx.enter_context(tc.tile_pool(name="dr", bufs=1, space="DRAM"))

        # ---- padded sequence: first DMA in flight ----
        t_seq = spool.tile([C, L + DC - 1], f16)
        nc.vector.memset(t_seq[:, 0:DC - 1], 0.0)
        nc.sync.dma_start(out=t_seq[:, DC - 1:], in_=seqT[:, :])

        # ---- weights (lo = p 0:128, hi = p 128:192) ----
        t_wc = wpool.tile([C, DC, P], f16)
        t_wz = wpool.tile([C, P], f16)
        t_wdtT = wpool.tile([DTR, P], f16)
        t_wsc = [wpool.tile([PLO, 2 + N], f32, name="wsc0"),
                 wpool.tile([PHI, 2 + N], f32, name="wsc1")]
        t_wbig0 = wpool.tile([PLO, W0 + 2 * C + PLO + PHI], f16)
        t_wbig1 = wpool.tile([PHI, W0 + 2 * C], f16)
        t_achi = wpool.tile([PLO, N // 2], f32)

        nc.sync.dma_start(out=t_wc, in_=wc[...])
        nc.sync.dma_start(out=t_wz, in_=wz[...])
        nc.sync.dma_start(out=t_wdtT, in_=wdtT[...])
        nc.sync.dma_start(out=t_wsc[0], in_=wsc[0:PLO, :])
        nc.sync.dma_start(out=t_wsc[1], in_=wsc[PLO:P, :])
        nc.sync.dma_start(out=t_wbig0, in_=wbig0[...])
        nc.sync.dma_start(out=t_wbig1, in_=wbig1[...])
        # hi pair-packed A columns: col j = [-A[128:192, 2j]; -A[128:192, 2j+1]]
        NW = 2 + N
        for par, off in ((0, 0), (1, 1)):
            nc.sync.dma_start(
                out=t_achi[par * PHI:(par + 1) * PHI, :],
                in_=bass.AP(tensor=wsc, offset=PLO * NW + 2 + off,
                            ap=[[NW, PHI], [2, N // 2]]))

        t_bdt = [t_wsc[0][:, 0:1], t_wsc[1][:, 0:1]]
        t_cb = [t_wsc[0][:, 1:2], t_wsc[1][:, 1:2]]
        t_aclo = t_wsc[0][:, 2:2 + N]
        t_wxT = [t_wbig0[:, 0:W0], t_wbig1[:, 0:W0]]
        t_woT = [t_wbig0[:, W0:W0 + C], t_wbig1[:, W0:W0 + C]]
        t_woT2 = [t_wbig0[:, W0 + C:W0 + 2 * C],
                  t_wbig1[:, W0 + C:W0 + 2 * C]]
        t_idsel = t_wbig0[:, W0 + 2 * C:]

        # persistent activations; hi-group dt/u replicated twice along partitions
        t_xa = [big_pool.tile([PLO, L], f16, name="xa0"),
                big_pool.tile([PHI, L], f16, name="xa1")]
        t_zs = [big_pool.tile([PLO, L], f16, name="zs0"),
                big_pool.tile([PHI, L], f16, name="zs1")]
        t_dt = [big_pool.tile([PLO, L], f32, name="dt0"),
                big_pool.tile([PLO, L], f32, name="dt1rep")]
        t_u = [big_pool.tile([PLO, L], f16, name="u0"),
               big_pool.tile([PLO, L], f16, name="u1rep")]
        t_proj = big_pool.tile([DTR + 2 * N, L], f16, name="proj")
        t_xz = [big_pool.tile([PLO, L], f16, name="xz0"),
                big_pool.tile([PHI, L], f16, name="xz1")]
        t_stlo = big_pool.tile([PLO, N], f32, name="stlo")
        t_sthi = big_pool.tile([PLO, N // 2], f32, name="sthi")

        # interleaved broadcast source: [n, section, B/C, col] so one item's
        # B||C block for a section is one contiguous 4KB run per partition
        bc_dram = dram_pool.tile([N, NSEC, 2, SEC], f16)

        PW = [PLO, PHI]

        # items: ("lo", n) x16 and ("hi", j) x8 (pair 2j, 2j+1)
        items = []
        for j in range(N // 2):
            items.append((0, 2 * j))
            items.append((0, 2 * j + 1))
            items.append((1, j))

        def emit_phase_c(si, ya_ps):
            s0 = si * SEC
            for q in range(SEC // MCH):
                g0 = s0 + q * MCH
                ps_o = ps_pool.tile([C, MCH], f32, tag="ps",
                                    name=f"pso_{si}_{q}")
                for i in range(2):
                    pw = PW[i]
                    ygz = tmp_pool.tile([PLO, MCH], f16, tag="ygz",
                                        name=f"ygz{i}_{si}_{q}")
                    nc.vector.tensor_tensor(
                        out=ygz[:pw],
                        in0=ya_ps[i][:pw, q * MCH:(q + 1) * MCH],
                        in1=t_zs[i][:, g0:g0 + MCH], op=Alu.mult)
                    nc.tensor.matmul(ps_o[:, :], t_woT[i], ygz[:pw, :],
                                     start=(i == 0), stop=False)
                    nc.tensor.matmul(ps_o[:, :], t_woT2[i],
                                     t_xz[i][:pw, g0:g0 + MCH],
                                     start=False, stop=(i == 1))
                o_sb = tmp_pool.tile([C, MCH], f32, tag="osb",
                                     name=f"osb{si}_{q}")
                if osb_act:
                    nc.scalar.activation(out=o_sb, in_=ps_o, func=Act.Copy)
                else:
                    nc.vector.tensor_copy(o_sb, ps_o)
                nc.sync.dma_start(out=out[:, g0:g0 + MCH], in_=o_sb)

        pending_c = []

        def body_half(half):
            """Phases A..C for one 1/halves slice of L."""
            GH = L // MCH // halves
            groups = range(half * GH, (half + 1) * GH)

            # ====== phase A: conv + silu-x, then z, then proj (Silu table) ==
            def emit_conv_x(s):
                g0 = s * MCH
                for i in range(2):
                    pw = PW[i]
                    ps_x = ps_pool.tile([PLO, MCH], f32, tag="ps",
                                        name=f"psx{i}_{s}")
                    for j in range(DC):
                        nc.tensor.matmul(ps_x[:pw, :],
                                         t_wc[:, j, i * PLO:i * PLO + pw],
                                         t_seq[:, g0 + j: g0 + j + MCH],
                                         start=(j == 0), stop=(j == DC - 1))
                    nc.scalar.activation(out=t_xa[i][:, g0:g0 + MCH],
                                         in_=ps_x[:pw], func=Act.Silu,
                                         bias=t_cb[i])

            def emit_z(s):
                g0 = s * MCH
                for i in range(2):
                    pw = PW[i]
                    ps_z = ps_pool.tile([PLO, MCH], f32, tag="ps",
                                        name=f"psz{i}_{s}")
                    nc.tensor.matmul(ps_z[:pw, :],
                                     t_wz[:, i * PLO:i * PLO + pw],
                                     t_seq[:, g0 + DC - 1: g0 + DC - 1 + MCH],
                                     start=True, stop=True)
                    nc.scalar.activation(out=t_zs[i][:, g0:g0 + MCH],
                                         in_=ps_z[:pw], func=Act.Silu)

            def emit_proj(s):
                g0 = s * MCH
                ps_proj = ps_pool.tile([DTR + 2 * N, MCH], f32, tag="ps",
                                       name=f"psp_{s}")
                for i in range(2):
                    nc.tensor.matmul(ps_proj[:, :], t_wxT[i],
                                     t_xa[i][:, g0:g0 + MCH],
                                     start=(i == 0), stop=(i == 1))
                if proj_act:
                    nc.scalar.activation(out=t_proj[:, g0:g0 + MCH],
                                         in_=ps_proj, func=Act.Copy)
                else:
                    nc.vector.tensor_copy(t_proj[:, g0:g0 + MCH], ps_proj)
                si, hf = s // 2, s % 2
                nc.sync.dma_start(
                    out=bc_dram[:, si, 0, hf * MCH:(hf + 1) * MCH],
                    in_=t_proj[DTR:DTR + N, g0:g0 + MCH])
                nc.sync.dma_start(
                    out=bc_dram[:, si, 1, hf * MCH:(hf + 1) * MCH],
                    in_=t_proj[DTR + N:, g0:g0 + MCH])

            for s in groups:
                emit_conv_x(s)
            for s in groups:
                emit_z(s)
            for s in groups:
                emit_proj(s)

            # ====== phase A3: dt via softplus(v) = -ln(sigmoid(-v)) ========
            # t_dt holds NEGATED dt; the sign is folded into the dA scale
            # (host passes +exp(A_log)) and the u multiply.
            for s in groups:
                g0 = s * MCH
                for i in range(2):
                    pw = PW[i]
                    ps_dt = ps_pool.tile([PLO, MCH], f32, tag="ps",
                                         name=f"psdt{i}_{s}")
                    nc.tensor.matmul(ps_dt[:pw, :],
                                     t_wdtT[:, i * PLO:i * PLO + pw],
                                     t_proj[0:DTR, g0:g0 + MCH],
                                     start=True, stop=True)
                    nc.scalar.activation(out=t_dt[i][:pw, g0:g0 + MCH],
                                         in_=ps_dt[:pw], func=Act.Sigmoid,
                                         scale=-1.0, bias=t_bdt[i])
            h0 = half * (L // halves)
            hsl = slice(h0, h0 + L // halves)
            for i in range(2):
                pw = PW[i]
                nc.scalar.activation(out=t_dt[i][:pw, hsl],
                                     in_=t_dt[i][:pw, hsl], func=Act.Ln)

            # ====== phase A4: u = dt * x, hi replication, xz ===============
            for i in range(2):
                pw = PW[i]
                nc.vector.scalar_tensor_tensor(
                    out=t_u[i][:pw, hsl], in0=t_dt[i][:pw, hsl], scalar=-1.0,
                    in1=t_xa[i][:, hsl], op0=Alu.mult, op1=Alu.mult)
            nc.gpsimd.tensor_copy(t_dt[1][PHI:PLO, hsl], t_dt[1][0:PHI, hsl])
            nc.vector.tensor_copy(t_u[1][PHI:PLO, hsl], t_u[1][0:PHI, hsl])
            for i in range(2):
                if xz_pool and pool_stt:
                    nc.gpsimd.scalar_tensor_tensor(
                        out=t_xz[i][:, hsl], in0=t_xa[i][:, hsl], scalar=1.0,
                        in1=t_zs[i][:, hsl], op0=Alu.mult, op1=Alu.mult)
                else:
                    nc.vector.tensor_tensor(out=t_xz[i][:, hsl],
                                            in0=t_xa[i][:, hsl],
                                            in1=t_zs[i][:, hsl], op=Alu.mult)

            # ====== phase B + deferred C per section (Exp table) ===========
            for si in range(half * (NSEC // halves),
                            (half + 1) * (NSEC // halves)):
                s0 = si * SEC
                ya_ps = [ya0_pool.tile([PLO, SEC], f32, tag="yaps0",
                                       name=f"yaps0_{si}"),
                         ya1_pool.tile([PHI, SEC], f32, tag="yaps1",
                                       name=f"yaps1_{si}")]
                seen = [0, 0]
                for it_idx, (g, n) in enumerate(items):
                    if pending_c and it_idx == interleave_c:
                        emit_phase_c(*pending_c.pop())
                    first = seen[g] == 0
                    seen[g] += 1
                    last = seen[g] == (N if g == 0 else N // 2)
                    # B||C rows for this item broadcast in one DMA
                    bcc = bc_pool.tile([PLO, 2, SEC], f16, tag="bcc",
                                       name=f"bcc_{si}_{g}_{n}")
                    nblk = NSEC * 2 * SEC
                    bcc_flat = bass.AP(tensor=bcc.tensor, offset=bcc.offset,
                                       ap=[[2 * SEC, PLO], [1, 2 * SEC]])
                    if g == 0:
                        nc.sync.dma_start(
                            out=bcc_flat, in_=bass.AP(
                                tensor=bc_dram.tensor,
                                offset=bc_dram.offset + n * nblk + si * 2 * SEC,
                                ap=[[0, PLO], [1, 2 * SEC]]))
                    else:
                        nc.sync.dma_start(
                            out=bcc_flat, in_=bass.AP(
                                tensor=bc_dram.tensor,
                                offset=(bc_dram.offset + 2 * n * nblk
                                        + si * 2 * SEC),
                                ap=[[nblk, 2], [0, PHI], [1, 2 * SEC]]))
                    bmb = bcc[:, 0, :]
                    cmb = bcc[:, 1, :]
                    acol_t = t_aclo if g == 0 else t_achi
                    st_t = t_stlo if g == 0 else t_sthi
                    dA = n_pool.tile([PLO, SEC], f16, tag="dA",
                                     name=f"dA_{si}_{g}_{n}")
                    nc.scalar.activation(out=dA, in_=t_dt[g][:, s0:s0 + SEC],
                                         func=Act.Exp,
                                         scale=acol_t[:, n:n + 1])
                    dBx = n_pool.tile([PLO, SEC], f16, tag="dBx",
                                      name=f"dBx_{si}_{g}_{n}")
                    if it_idx >= 24 - dbx_pool:
                        if pool_stt:
                            nc.gpsimd.scalar_tensor_tensor(
                                out=dBx, in0=t_u[g][:, s0:s0 + SEC],
                                scalar=1.0, in1=bmb,
                                op0=Alu.mult, op1=Alu.mult)
                        else:
                            nc.gpsimd.tensor_tensor(
                                out=dBx, in0=t_u[g][:, s0:s0 + SEC],
                                in1=bmb, op=Alu.mult)
                    else:
                        nc.vector.tensor_tensor(
                            out=dBx, in0=t_u[g][:, s0:s0 + SEC],
                            in1=bmb, op=Alu.mult)
                    h = n_pool.tile([PLO, SEC], f16, tag="h",
                                    name=f"h_{si}_{g}_{n}")
                    nc.vector.tensor_tensor_scan(
                        out=h, data0=dA, data1=dBx,
                        initial=st_t[:, n:n + 1],
                        op0=Alu.mult, op1=Alu.add)
                    nc.vector.tensor_copy(st_t[:, n:n + 1], h[:, SEC - 1:SEC])
                    hc = n_pool.tile([PLO, SEC], f16, tag="hc",
                                     name=f"hc_{si}_{g}_{n}")
                    if it_idx < 24 - hc_pool:
                        nc.vector.tensor_tensor(out=hc, in0=h, in1=cmb,
                                                op=Alu.mult)
                    elif pool_stt:
                        nc.gpsimd.scalar_tensor_tensor(
                            out=hc, in0=h, scalar=1.0, in1=cmb,
                            op0=Alu.mult, op1=Alu.mult)
                    else:
                        nc.gpsimd.tensor_tensor(out=hc, in0=h, in1=cmb,
                                                op=Alu.mult)
                    lhs = (t_idsel[:, 0:PLO] if g == 0
                           else t_idsel[:, PLO:PLO + PHI])
                    for q in range(SEC // MCH):
                        nc.tensor.matmul(
                            ya_ps[g][:, q * MCH:(q + 1) * MCH], lhs,
                            hc[:, q * MCH:(q + 1) * MCH],
                            start=first, stop=last)
                if interleave_c is None or interleave_c < 0:
                    emit_phase_c(si, ya_ps)
                else:
                    pending_c.append((si, ya_ps))

        def body(_iv=None):
            nc.vector.memset(t_stlo, 0.0)
            nc.vector.memset(t_sthi, 0.0)
            for hf in range(halves):
                body_half(hf)
            if pending_c:
                emit_phase_c(*pending_c.pop())

        if repeat == 1:
            body()
        else:
            with tc.For_i(0, repeat, 1) as iv:
                body(iv)

    nc.compile()
    return nc


def _prep_core_inputs(inp, d, seqT):
    W_in = inp['W_in'][d]
    conv_w = inp['conv_w'][d]
    negA = np.exp(inp['A_log'][d])                          # -A = +exp(A_log)
    wc = np.einsum('pc,pj->cjp', W_in[:P, :], conv_w)       # (C, DC, P)
    idsel = np.concatenate(
        [np.eye(PLO, dtype=np.float16),
         np.vstack([np.eye(PHI, dtype=np.float16)] * 2)], axis=1)
    wxT = inp['W_x'][d].T.astype(np.float16)                # (P, 38)
    woT = inp['W_out'][d].T.astype(np.float16)              # (P, C)
    woT2 = (inp['W_out'][d].T * inp['Dp'][d][:, None]).astype(np.float16)
    wsc = np.concatenate([-inp['b_dt'][d][:, None],
                          inp['conv_b'][d][:, None],
                          negA], axis=1).astype(np.float32)  # (P, 2+N)
    wbig0 = np.concatenate([wxT[:PLO], woT[:PLO], woT2[:PLO], idsel],
                           axis=1)                           # (128, 422)
    wbig1 = np.concatenate([wxT[PLO:], woT[PLO:], woT2[PLO:]],
                           axis=1)                           # (64, 230)
    return {
        'seqT': np.ascontiguousarray(seqT).astype(np.float16),
        'wc': np.ascontiguousarray(wc).astype(np.float16),
        'wz': np.ascontiguousarray(W_in[P:, :].T).astype(np.float16),
        'wdtT': np.ascontiguousarray(inp['W_dt'][d].T).astype(np.float16),
        'wsc': np.ascontiguousarray(wsc),
        'wbig0': np.ascontiguousarray(wbig0),
        'wbig1': np.ascontiguousarray(wbig1),
    }


def kernel(x1, x2, W_in, conv_w, conv_b, W_x, W_dt, b_dt, A_log, Dp, W_out):
    from concourse.bass_utils import run_bass_kernel_spmd

    inp = dict(x1=np.asarray(x1), x2=np.asarray(x2), W_in=np.asarray(W_in),
               conv_w=np.asarray(conv_w), conv_b=np.asarray(conv_b),
               W_x=np.asarray(W_x), W_dt=np.asarray(W_dt),
               b_dt=np.asarray(b_dt), A_log=np.asarray(A_log),
               Dp=np.asarray(Dp), W_out=np.asarray(W_out))
    B = inp['x1'].shape[0]

    if 'nc' not in _CACHED:
        _CACHED['nc'] = _build_program()
    nc = _CACHED['nc']

    in_maps = []
    metas = []
    for d in range(4):
        for b in range(B):
            x = inp['x1'][b]
            if d < 2:
                seq = x.reshape(C, L)
            else:
                seq = np.ascontiguousarray(x.transpose(0, 2, 1)).reshape(C, L)
            if d in (1, 3):
                seq = seq[:, ::-1]
            in_maps.append(_prep_core_inputs(inp, d, seq))
            metas.append((d, b))

    res = run_bass_kernel_spmd(nc, in_maps, core_ids=list(range(len(in_maps))))

    outs = np.zeros((B, C, HH, WW), np.float32)
    for (d, b), r in zip(metas, res.results):
        y = r['out']                      # (C, L)
        if d in (1, 3):
            y = y[:, ::-1]
        if d < 2:
            y = y.reshape(C, HH, WW)
        else:
            y = y.reshape(C, WW, HH).transpose(0, 2, 1)
        outs[b] += y
    outs += inp['x2']
    return outs
